# revision 15
# baseline (speedup 1.0000x reference)
"""Multi-head attention (B=2, S=2048, D=2048, H=16) on 8 TRN2 NeuronCores.

Sharding: data-parallel over batch (2) x Megatron tensor-parallel over heads
(4 groups of 4 heads). Core c = 4*b + g handles batch b, heads [4g, 4g+4).
Each core computes q/k/v projections for its head slice, attention over its
4 heads, and a partial o_proj contribution; the host sums the 4 partials per
batch (the unshard step of Megatron TP) and stacks the 2 batches.

Precision: projections and o_proj run in bf16 (x, wq/wk/wv, kT, qT, ctx, wo);
attention-value and softmax stay in f32r/f32 (vv, expP, denominators). All
matmul accumulation is fp32 in PSUM. Measured max-rel-err ~6e-3 vs the fp32
reference (harness gate 2e-2).

Schedule (single pass, everything SBUF-resident, no DRAM spills):
  Prefix: per 512-seq chunk {k-projection -> kT, v-projection -> vv}.
  QA (per 512-query chunk): a per-head kt-loop {scores matmul, exp (ACT),
  pairwise denominator accumulate (DVE/Pool alternating), attn@v accumulate}
  with one independent matmul interleaved per iteration so the in-order
  tensor-engine queue never drains: chunk 0 interleaves chunk 1's
  q-projection legs, chunks 1-3 interleave the PREVIOUS chunk's o_proj legs.
  Per-head normalization (colsum ones-matmul on fp32r-bitcast accumulators,
  reciprocal_approx_fast, partition_broadcast, multiply) is software-
  pipelined one head late so its cross-engine dependency chain resolves
  behind ~4us of queued PE work instead of stalling the PE.

Why: the PE is the roofline engine (~331us of unavoidable matmul rows per
core at 2.4 GHz) and the HAM clock gate halves its clock after ~3.4us of
idle, so the whole design optimizes for an unbroken PE instruction stream.
"""

import contextlib
import math
import os

import numpy as np
import ml_dtypes

import concourse.mybir as mybir
import concourse.tile as tile
from concourse import bacc
from concourse.bass_utils import run_bass_kernel_spmd

F32 = mybir.dt.float32
F32R = mybir.dt.float32r
BF16 = mybir.dt.bfloat16

B, S, D = 2, 2048, 2048
H = 16
HD = 128
G = 4                    # tensor-parallel groups
HLOC = H // G            # heads per core = 4
DG = HLOC * HD           # per-core projection width = 512
P = 128
NCORES = 8

SCHUNK = 512             # seq chunk
NSC = S // SCHUNK        # 4 chunks
ST = SCHUNK // P         # 4 s-tiles per chunk
DT = D // P              # 16 contraction tiles
MT = DG // P             # 4 output tiles (== HLOC)
KT = S // P              # 16 key tiles
IC = D // SCHUNK         # 4 o_proj output column blocks
INV_SQRT_HD = 1.0 / math.sqrt(HD)

_cache = {}
last_run = None  # BassKernelResults of the most recent execution (for test.py)


def build(loop_reps=None):
    nc = bacc.Bacc(None, target_bir_lowering=False)

    xT_dr = nc.dram_tensor("xT", [D, S], BF16, kind="ExternalInput")
    wqT_dr = nc.dram_tensor("wqT", [D, DG], BF16, kind="ExternalInput")
    wkT_dr = nc.dram_tensor("wkT", [D, DG], BF16, kind="ExternalInput")
    wvT_dr = nc.dram_tensor("wvT", [D, DG], BF16, kind="ExternalInput")
    woT_dr = nc.dram_tensor("woT", [DG, D], BF16, kind="ExternalInput")
    out_d = nc.dram_tensor("out", [S, D], F32, kind="ExternalOutput")

    xT_view = xT_dr.rearrange("(o p) s -> p o s", p=P)
    wkT_v = wkT_dr.rearrange("(o p) m -> p o m", p=P)
    wqT_v = wqT_dr.rearrange("(o p) m -> p o m", p=P)
    wvT_v = wvT_dr.rearrange("(o p) m -> p o m", p=P)
    woT_v = woT_dr.rearrange("(o p) i -> p o i", p=P)

    with tile.TileContext(nc) as tc:
        loop_cm = tc.For_i(0, loop_reps, 1) if loop_reps else contextlib.nullcontext()
        with loop_cm:
            with (
                tc.tile_pool(name="persist", bufs=1) as pers,
                tc.tile_pool(name="xring", bufs=3) as xring,
            ):
                kT = pers.tile([P, HLOC, S], BF16, tag="kT")
                vv = pers.tile([P, NSC, ST, HLOC, HD], F32R, tag="vv")
                wq = pers.tile([P, DT, DG], BF16, tag="wq")
                wo = pers.tile([P, MT, D], BF16, tag="wo")
                ones = pers.tile([P, 1], F32R, tag="ones")
                ones_f = pers.tile([P, 1], F32, tag="ones_f")
                nc.vector.memset(ones_f[:], 1.0)
                nc.vector.tensor_copy(ones[:], ones_f[:])

                def load_x(sc):
                    xt = xring.tile([P, DT, SCHUNK], BF16, tag="x")
                    c0 = sc * SCHUNK
                    for d0 in range(0, DT, 4):
                        nc.sync.dma_start(
                            xt[:, d0:d0 + 4], xT_view[:, d0:d0 + 4, c0:c0 + SCHUNK])
                    return xt

                # ---------- prefix: k/v projections, SBUF-resident ----------
                with (
                    tc.tile_pool(name="wkv", bufs=1) as wkvp,
                    tc.tile_pool(name="psumP", bufs=1, space="PSUM") as psP,
                ):
                    wk = wkvp.tile([P, DT, DG], BF16, tag="wk")
                    wv = wkvp.tile([P, DT, DG], BF16, tag="wv")
                    # DMA order matters: the first k-matmul needs wk + x(0),
                    # so x(0) goes right after wk, ahead of wv/wq/wo
                    for d0 in range(0, DT, 4):
                        nc.sync.dma_start(wk[:, d0:d0 + 4], wkT_v[:, d0:d0 + 4])
                    xt0 = load_x(0)
                    for d0 in range(0, DT, 4):
                        nc.sync.dma_start(wv[:, d0:d0 + 4], wvT_v[:, d0:d0 + 4])
                    for d0 in range(0, DT, 4):
                        nc.sync.dma_start(wq[:, d0:d0 + 4], wqT_v[:, d0:d0 + 4])
                    for j0 in range(MT):
                        nc.sync.dma_start(wo[:, j0:j0 + 1], woT_v[:, j0:j0 + 1])

                    for sc in range(NSC):
                        xt = xt0 if sc == 0 else load_x(sc)
                        c0 = sc * SCHUNK
                        for mt in range(MT):
                            ps = psP.tile([P, SCHUNK], F32, tag="kps", bufs=3)
                            for dt in range(DT):
                                nc.tensor.matmul(
                                    ps[:], wk[:, dt, mt * P:(mt + 1) * P],
                                    xt[:, dt, :],
                                    start=(dt == 0), stop=(dt == DT - 1))
                            nc.vector.tensor_copy(kT[:, mt, c0:c0 + SCHUNK], ps[:])
                        for st in range(ST):
                            ps = psP.tile([P, DG], F32, tag="vps", bufs=3)
                            for dt in range(DT):
                                nc.tensor.matmul(
                                    ps[:], xt[:, dt, st * P:(st + 1) * P],
                                    wv[:, dt, :],
                                    start=(dt == 0), stop=(dt == DT - 1))
                            nc.vector.tensor_copy(vv[:, sc, st], ps[:])

                # ---------- QA: per query chunk ----------
                with (
                    tc.tile_pool(name="qts", bufs=2) as qts,
                    tc.tile_pool(name="ctxp", bufs=2) as ctxp,
                    tc.tile_pool(name="expp", bufs=4) as expp,
                    tc.tile_pool(name="accp", bufs=2) as accp,
                    tc.tile_pool(name="small", bufs=2) as small,
                    tc.tile_pool(name="psumQ", bufs=1, space="PSUM") as psQ,
                ):
                    # --- emit helpers -------------------------------------
                    def qproj_leg(qTs_t, xt, i, state):
                        """q-projection leg i (0..63): mt = i//16, dt = i%16."""
                        mt, dt = divmod(i, DT)
                        if dt == 0:
                            state["qp"] = psQ.tile([P, SCHUNK], F32, tag="op",
                                                   bufs=2, name="qpps")
                        ps = state["qp"]
                        nc.tensor.matmul(
                            ps[:], wq[:, dt, mt * P:(mt + 1) * P], xt[:, dt, :],
                            start=(dt == 0), stop=(dt == DT - 1))
                        if dt == DT - 1:
                            nc.scalar.copy(qTs_t[:, mt, :], ps[:])

                    def oproj_leg(ctx_t, pqc, i, state):
                        """o_proj leg i (0..63) of chunk pqc: chain=i//4."""
                        chain, leg = divmod(i, MT)
                        st, ic = divmod(chain, IC)
                        if leg == 0:
                            state["op"] = psQ.tile([P, SCHUNK], F32, tag="op",
                                                   bufs=2, name="opps")
                        ps = state["op"]
                        nc.tensor.matmul(
                            ps[:], ctx_t[:, leg, st * P:(st + 1) * P],
                            wo[:, leg, ic * SCHUNK:(ic + 1) * SCHUNK],
                            start=(leg == 0), stop=(leg == MT - 1))
                        if leg == MT - 1:
                            ob = small.tile([P, SCHUNK], F32, tag="ob", bufs=3)
                            if chain % 4 == 0:
                                nc.scalar.copy(ob[:], ps[:])
                            else:
                                nc.vector.tensor_copy(ob[:], ps[:])
                            r0 = pqc * SCHUNK + st * P
                            nc.sync.dma_start(
                                out_d[r0:r0 + P, ic * SCHUNK:(ic + 1) * SCHUNK],
                                ob[:])

                    # The per-head normalization is emitted in PIECES so
                    # each cross-engine hop (DVE merge -> PE colsum -> DVE
                    # recip -> Pool bcast -> DVE mul) resolves behind queued
                    # PE work, and no productive engine op ever sits behind a
                    # chain wait in its in-order queue.
                    def norm_merge(state, eng=None):
                        accA, accB, _, _, _ = state["pend"]
                        # cast-on-write merge: the BIR verifier requires f32r
                        # matmul operands to be f32r-rounded by their producer.
                        # Runs on the Pool engine by default: Pool is idle in
                        # the first pairs of each head, while the DVE backlog
                        # was delaying the merge (and the colsum with it).
                        macc = accp.tile([P, 2, SCHUNK], F32R, tag="macc",
                                         name="macc")
                        (eng or nc.gpsimd).tensor_add(macc[:], accA[:], accB[:])
                        state["macc"] = macc

                    def norm_colsum(state):
                        # colsum lands in row 0 of an op-ring psum tile so the
                        # pss pair tiles can have the spare PSUM bank
                        macc = state["macc"]
                        psd = psQ.tile([P, SCHUNK], F32, tag="op", bufs=2,
                                       name="psden")
                        nc.tensor.matmul(
                            psd[0:1, :], ones[:], macc[:, 0, :],
                            start=True, stop=False)
                        nc.tensor.matmul(
                            psd[0:1, :], ones[:], macc[:, 1, :],
                            start=False, stop=True)
                        recip = small.tile([1, SCHUNK], F32, tag="recip")
                        nc.vector.reciprocal_approx_fast(recip[:], psd[0:1, :])
                        state["recip"] = recip

                    def norm_bcast(state):
                        rb = small.tile([P, SCHUNK], F32, tag="rb")
                        nc.gpsimd.partition_broadcast(rb[:], state["recip"][:])
                        state["rb"] = rb

                    def norm_mul(state):
                        _, _, pso, ctx_t, h = state["pend"]
                        nc.vector.tensor_mul(ctx_t[:, h, :], pso[:],
                                             state["rb"][:])
                        state["pend"] = None

                    def h_loop(qc, qTs_t, ctx_t, filler, state):
                        """Scores/softmax/attnv for all 4 heads of chunk qc.
                        `filler(i, state)` emits one independent PE matmul per
                        iteration i = h*KT + kt (64 total). The normalization
                        of head h-1 is emitted at head h's kt==6; head 3's is
                        left in state["pend"] for the next chunk's prologue."""
                        for h in range(HLOC):
                            accA = accp.tile([P, 2, SCHUNK], F32, tag="accA")
                            accB = accp.tile([P, 2, SCHUNK], F32, tag="accB")
                            pso = psQ.tile([P, SCHUNK], F32, tag="pso", bufs=2)
                            pairs = []
                            for pi in range(KT // 2):
                                kt0 = 2 * pi
                                # two scores matmuls into one 2-bank psum
                                # pair, then ONE 1024-wide exp (halves the
                                # ACT instruction count)
                                pss = psQ.tile([P, 2, SCHUNK], F32, tag="pss",
                                               bufs=2, name="pss")
                                for j in range(2):
                                    nc.tensor.matmul(
                                        pss[:, j, :],
                                        kT[:, h, (kt0 + j) * P:(kt0 + j + 1) * P],
                                        qTs_t[:, h, :], start=True, stop=True)
                                ep = expp.tile([P, 2, SCHUNK], F32R,
                                               tag="expP", bufs=5, name="ep")
                                pairs.append(ep)
                                nc.scalar.activation(
                                    ep[:], pss[:],
                                    mybir.ActivationFunctionType.Exp,
                                    scale=INV_SQRT_HD)
                                for j in range(2):
                                    kt = kt0 + j
                                    nc.tensor.matmul(
                                        pso[:], vv[:, kt // ST, kt % ST, h, :],
                                        ep[:, j, :],
                                        start=(kt == 0), stop=(kt == KT - 1))
                                    filler(h * KT + kt, state)
                                # Denominator accumulate: DVE takes pairs
                                # {0,2,4,6,7}, Pool {1,3,5} -- the slow Q7
                                # Pool never gates the merge. Head h-1's norm
                                # pieces are spaced so every hop resolves
                                # behind queued PE work (see norm_* above).
                                pend = state["pend"]
                                if pi == 1 and pend is not None:
                                    norm_merge(state)
                                elif pi == 2:
                                    nc.vector.tensor_add(
                                        accA[:], pairs[0][:].bitcast(F32),
                                        pairs[2][:].bitcast(F32))
                                elif pi == 3:
                                    nc.gpsimd.tensor_add(
                                        accB[:], pairs[1][:].bitcast(F32),
                                        pairs[3][:].bitcast(F32))
                                elif pi == 4:
                                    nc.vector.tensor_add(
                                        accA[:], accA[:],
                                        pairs[4][:].bitcast(F32))
                                elif pi == 5:
                                    nc.gpsimd.tensor_add(
                                        accB[:], accB[:],
                                        pairs[5][:].bitcast(F32))
                                elif pi == 6:
                                    nc.vector.tensor_add(
                                        accA[:], accA[:],
                                        pairs[6][:].bitcast(F32))
                                    if pend is not None:
                                        norm_colsum(state)
                                        norm_bcast(state)
                                elif pi == 7:
                                    nc.vector.tensor_add(
                                        accA[:], accA[:],
                                        pairs[7][:].bitcast(F32))
                                    if pend is not None:
                                        norm_mul(state)
                            state["pend"] = (accA, accB, pso, ctx_t, h)

                    # --- QA schedule --------------------------------------
                    xts = [None] * NSC
                    qTs_l = [None] * NSC
                    ctxs = [None] * NSC
                    state = {"pend": None}
                    nofill = lambda i, st: None

                    xts[0] = load_x(0)
                    xts[1] = load_x(1)
                    # q-projection block for chunk 0 (PE filler: none needed,
                    # the PE is saturated by the block itself)
                    qTs_l[0] = qts.tile([P, HLOC, SCHUNK], BF16, tag="qT", name="qT0")
                    st0 = {}
                    for i in range(MT * DT):
                        qproj_leg(qTs_l[0], xts[0], i, st0)

                    for qc in range(NSC):
                        ctxs[qc] = ctxp.tile([P, HLOC, SCHUNK], BF16, tag="ctx", name="ctx")
                        if qc == 0:
                            # interleave chunk 1's qproj legs mt=0..2
                            xts[2] = load_x(2)
                            qTs_l[1] = qts.tile([P, HLOC, SCHUNK], BF16,
                                                tag="qT", name="qT1")
                            filler = (lambda i, st:
                                      qproj_leg(qTs_l[1], xts[1], i, st)
                                      if i < 48 else None)
                        else:
                            # prologue: finish/run this chunk's qproj, with
                            # the previous chunk's head-3 norm pieces spaced
                            # through it (psden allocated after the second
                            # chain so the op-ring phase stays clean)
                            if qc == 1:
                                xts[3] = load_x(3)
                                for i in range(48, 64):
                                    qproj_leg(qTs_l[1], xts[1], i, state)
                                    if i == 49:
                                        norm_merge(state)
                                    elif i == 54:
                                        norm_colsum(state)
                                    elif i == 58:
                                        norm_bcast(state)
                                    elif i == 61:
                                        norm_mul(state)
                            else:
                                qTs_l[qc] = qts.tile([P, HLOC, SCHUNK], BF16,
                                                     tag="qT", name="qTn")
                                for i in range(MT * DT):
                                    qproj_leg(qTs_l[qc], xts[qc], i, state)
                                    if i == 1:
                                        norm_merge(state)
                                    elif i == DT + 1:
                                        norm_colsum(state)
                                    elif i == DT + 5:
                                        norm_bcast(state)
                                    elif i == DT + 9:
                                        norm_mul(state)
                            pctx = ctxs[qc - 1]
                            pqc = qc - 1
                            filler = (lambda i, st, c=pctx, q=pqc:
                                      oproj_leg(c, q, i, st))
                        h_loop(qc, qTs_l[qc], ctxs[qc], filler, state)

                    # tail: o_proj for the last chunk. Chain 0's jt=0..2
                    # legs need only ctx[0..2], so they cover the last head's
                    # norm pieces; its jt=3 leg follows the multiply.
                    norm_merge(state)
                    st_a = {}
                    for leg in range(MT - 1):
                        oproj_leg(ctxs[NSC - 1], NSC - 1, leg, st_a)
                    norm_colsum(state)
                    norm_bcast(state)
                    norm_mul(state)
                    oproj_leg(ctxs[NSC - 1], NSC - 1, MT - 1, st_a)
                    for i in range(MT, HLOC * KT):
                        oproj_leg(ctxs[NSC - 1], NSC - 1, i, state)

    nc.finalize()
    return nc

_build = build


def _bf16(a):
    return np.ascontiguousarray(a, dtype=np.float32).astype(ml_dtypes.bfloat16)


def kernel(hidden_states, wq, wk, wv, wo):
    global last_run
    if "nc" not in _cache:
        _cache["nc"] = build()
    nc = _cache["nc"]

    hidden_states = np.asarray(hidden_states, dtype=np.float32)
    wq = np.asarray(wq, dtype=np.float32)
    wk = np.asarray(wk, dtype=np.float32)
    wv = np.asarray(wv, dtype=np.float32)
    wo = np.asarray(wo, dtype=np.float32)

    xT = [_bf16(hidden_states[b].T) for b in range(B)]
    in_maps = []
    for c in range(NCORES):
        b, g = divmod(c, G)
        sl = slice(g * DG, (g + 1) * DG)
        in_maps.append({
            "xT": xT[b],
            "wqT": _bf16(wq[sl, :].T),
            "wkT": _bf16(wk[sl, :].T),
            "wvT": _bf16(wv[sl, :].T),
            "woT": _bf16(wo[:, sl].T),
        })

    trace = os.environ.get("BASSKERNEL_TRACE", "0") == "1"
    last_run = run_bass_kernel_spmd(
        nc, in_maps, core_ids=list(range(NCORES)), trace=trace)

    out = np.empty((B, S, D), dtype=np.float32)
    for b in range(B):
        acc = None
        for g in range(G):
            part = last_run.results[b * G + g]["out"]
            acc = part.copy() if acc is None else acc + part
        out[b] = acc
    return out


# revision 16
# speedup vs baseline: 1.1210x; 1.1210x over previous
"""Multi-head attention (B=2, S=2048, D=2048, H=16) on 8 TRN2 NeuronCores.

Sharding: data-parallel over batch (2) x Megatron tensor-parallel over heads
(4 groups of 4 heads). Core c = 4*b + g handles batch b, heads [4g, 4g+4).
Each core computes q/k/v projections for its head slice, attention over its
4 heads, and a partial o_proj contribution; the host sums the 4 partials per
batch (the unshard step of Megatron TP) and stacks the 2 batches.

Precision: projections and o_proj run in bf16 (x, wq/wk/wv, kT, qT, ctx, wo);
attention-value and softmax stay in f32r/f32 (vv, expP, denominators). All
matmul accumulation is fp32 in PSUM. Measured max-rel-err ~6e-3 vs the fp32
reference (harness gate 2e-2).

Schedule (single pass, everything SBUF-resident, no DRAM spills):
  Prefix: per 512-seq chunk {k-projection -> kT, v-projection -> vv}.
  QA (per 512-query chunk): a per-head kt-loop {scores matmul, exp (ACT),
  pairwise denominator accumulate (DVE/Pool alternating), attn@v accumulate}
  with one independent matmul interleaved per iteration so the in-order
  tensor-engine queue never drains: chunk 0 interleaves chunk 1's
  q-projection legs, chunks 1-3 interleave the PREVIOUS chunk's o_proj legs.
  Per-head normalization (colsum ones-matmul on fp32r-bitcast accumulators,
  reciprocal_approx_fast, partition_broadcast, multiply) is software-
  pipelined one head late so its cross-engine dependency chain resolves
  behind ~4us of queued PE work instead of stalling the PE.

Why: the PE is the roofline engine (~331us of unavoidable matmul rows per
core at 2.4 GHz) and the HAM clock gate halves its clock after ~3.4us of
idle, so the whole design optimizes for an unbroken PE instruction stream.
"""

import contextlib
import math
import os

import numpy as np
import ml_dtypes

import concourse.mybir as mybir
import concourse.tile as tile
from concourse import bacc
from concourse.bass_utils import run_bass_kernel_spmd

F32 = mybir.dt.float32
F32R = mybir.dt.float32r
BF16 = mybir.dt.bfloat16

B, S, D = 2, 2048, 2048
H = 16
HD = 128
G = 4                    # tensor-parallel groups
HLOC = H // G            # heads per core = 4
DG = HLOC * HD           # per-core projection width = 512
P = 128
NCORES = 8

SCHUNK = 512             # seq chunk
NSC = S // SCHUNK        # 4 chunks
ST = SCHUNK // P         # 4 s-tiles per chunk
DT = D // P              # 16 contraction tiles
MT = DG // P             # 4 output tiles (== HLOC)
KT = S // P              # 16 key tiles
IC = D // SCHUNK         # 4 o_proj output column blocks
INV_SQRT_HD = 1.0 / math.sqrt(HD)

_cache = {}
last_run = None  # BassKernelResults of the most recent execution (for test.py)


def build(loop_reps=None):
    nc = bacc.Bacc(None, target_bir_lowering=False)

    xT_dr = nc.dram_tensor("xT", [D, S], BF16, kind="ExternalInput")
    wqT_dr = nc.dram_tensor("wqT", [D, DG], BF16, kind="ExternalInput")
    wkT_dr = nc.dram_tensor("wkT", [D, DG], BF16, kind="ExternalInput")
    wvT_dr = nc.dram_tensor("wvT", [D, DG], BF16, kind="ExternalInput")
    woT_dr = nc.dram_tensor("woT", [DG, D], BF16, kind="ExternalInput")
    out_d = nc.dram_tensor("out", [S, D], F32, kind="ExternalOutput")

    xT_view = xT_dr.rearrange("(o p) s -> p o s", p=P)
    wkT_v = wkT_dr.rearrange("(o p) m -> p o m", p=P)
    wqT_v = wqT_dr.rearrange("(o p) m -> p o m", p=P)
    wvT_v = wvT_dr.rearrange("(o p) m -> p o m", p=P)
    woT_v = woT_dr.rearrange("(o p) i -> p o i", p=P)

    with tile.TileContext(nc) as tc:
        loop_cm = tc.For_i(0, loop_reps, 1) if loop_reps else contextlib.nullcontext()
        with loop_cm:
            with (
                tc.tile_pool(name="persist", bufs=1) as pers,
                tc.tile_pool(name="xring", bufs=3) as xring,
            ):
                kT = pers.tile([P, HLOC, S], BF16, tag="kT")
                vv = pers.tile([P, NSC, ST, HLOC, HD], F32R, tag="vv")
                wq = pers.tile([P, DT, DG], BF16, tag="wq")
                wo = pers.tile([P, MT, D], BF16, tag="wo")
                ones = pers.tile([P, 1], F32R, tag="ones")
                ones_f = pers.tile([P, 1], F32, tag="ones_f")
                nc.vector.memset(ones_f[:], 1.0)
                nc.vector.tensor_copy(ones[:], ones_f[:])

                def load_x(sc):
                    xt = xring.tile([P, DT, SCHUNK], BF16, tag="x")
                    c0 = sc * SCHUNK
                    for d0 in range(0, DT, 4):
                        nc.sync.dma_start(
                            xt[:, d0:d0 + 4], xT_view[:, d0:d0 + 4, c0:c0 + SCHUNK])
                    return xt

                # ---------- prefix: k/v projections, SBUF-resident ----------
                with (
                    tc.tile_pool(name="wkv", bufs=1) as wkvp,
                    tc.tile_pool(name="psumP", bufs=1, space="PSUM") as psP,
                ):
                    wk = wkvp.tile([P, DT, DG], BF16, tag="wk")
                    wv = wkvp.tile([P, DT, DG], BF16, tag="wv")
                    # DMA order matters: the first k-matmul needs wk + x(0),
                    # so x(0) goes right after wk, ahead of wv/wq/wo
                    for d0 in range(0, DT, 4):
                        nc.sync.dma_start(wk[:, d0:d0 + 4], wkT_v[:, d0:d0 + 4])
                    xt0 = load_x(0)
                    for d0 in range(0, DT, 4):
                        nc.sync.dma_start(wv[:, d0:d0 + 4], wvT_v[:, d0:d0 + 4])
                    for d0 in range(0, DT, 4):
                        nc.sync.dma_start(wq[:, d0:d0 + 4], wqT_v[:, d0:d0 + 4])
                    for j0 in range(MT):
                        nc.sync.dma_start(wo[:, j0:j0 + 1], woT_v[:, j0:j0 + 1])

                    for sc in range(NSC):
                        xt = xt0 if sc == 0 else load_x(sc)
                        c0 = sc * SCHUNK
                        for mt in range(MT):
                            ps = psP.tile([P, SCHUNK], F32, tag="kps", bufs=3)
                            for dt in range(DT):
                                nc.tensor.matmul(
                                    ps[:], wk[:, dt, mt * P:(mt + 1) * P],
                                    xt[:, dt, :],
                                    start=(dt == 0), stop=(dt == DT - 1))
                            nc.vector.tensor_copy(kT[:, mt, c0:c0 + SCHUNK], ps[:])
                        for st in range(ST):
                            ps = psP.tile([P, DG], F32, tag="vps", bufs=3)
                            for dt in range(DT):
                                nc.tensor.matmul(
                                    ps[:], xt[:, dt, st * P:(st + 1) * P],
                                    wv[:, dt, :],
                                    start=(dt == 0), stop=(dt == DT - 1))
                            nc.vector.tensor_copy(vv[:, sc, st], ps[:])

                # ---------- QA: per query chunk ----------
                with (
                    tc.tile_pool(name="qts", bufs=2) as qts,
                    tc.tile_pool(name="ctxp", bufs=2) as ctxp,
                    tc.tile_pool(name="expp", bufs=4) as expp,
                    tc.tile_pool(name="accp", bufs=2) as accp,
                    tc.tile_pool(name="small", bufs=2) as small,
                    tc.tile_pool(name="psumQ", bufs=1, space="PSUM") as psQ,
                ):
                    # --- emit helpers -------------------------------------
                    def qproj_leg(qTs_t, xt, i, state):
                        """q-projection leg i (0..63): mt = i//16, dt = i%16."""
                        mt, dt = divmod(i, DT)
                        if dt == 0:
                            state["qp"] = psQ.tile([P, SCHUNK], F32, tag="op",
                                                   bufs=2, name="qpps")
                        ps = state["qp"]
                        nc.tensor.matmul(
                            ps[:], wq[:, dt, mt * P:(mt + 1) * P], xt[:, dt, :],
                            start=(dt == 0), stop=(dt == DT - 1))
                        if dt == DT - 1:
                            nc.scalar.copy(qTs_t[:, mt, :], ps[:])

                    def oproj_leg(ctx_t, pqc, i, state):
                        """o_proj leg i (0..63) of chunk pqc: chain=i//4."""
                        chain, leg = divmod(i, MT)
                        st, ic = divmod(chain, IC)
                        if leg == 0:
                            state["op"] = psQ.tile([P, SCHUNK], F32, tag="op",
                                                   bufs=2, name="opps")
                        ps = state["op"]
                        nc.tensor.matmul(
                            ps[:], ctx_t[:, leg, st * P:(st + 1) * P],
                            wo[:, leg, ic * SCHUNK:(ic + 1) * SCHUNK],
                            start=(leg == 0), stop=(leg == MT - 1))
                        if leg == MT - 1:
                            ob = small.tile([P, SCHUNK], F32, tag="ob", bufs=3)
                            if chain % 4 == 0:
                                nc.scalar.copy(ob[:], ps[:])
                            else:
                                nc.vector.tensor_copy(ob[:], ps[:])
                            r0 = pqc * SCHUNK + st * P
                            nc.sync.dma_start(
                                out_d[r0:r0 + P, ic * SCHUNK:(ic + 1) * SCHUNK],
                                ob[:])

                    # The per-head normalization is emitted in PIECES so
                    # each cross-engine hop (DVE merge -> PE colsum -> DVE
                    # recip -> Pool bcast -> DVE mul) resolves behind queued
                    # PE work, and no productive engine op ever sits behind a
                    # chain wait in its in-order queue.
                    def norm_merge(state):
                        accA, accB, _, _, _ = state["pend"]
                        # cast-on-write merge: the BIR verifier requires f32r
                        # matmul operands to be f32r-rounded by their producer
                        macc = accp.tile([P, 2, SCHUNK], F32R, tag="macc",
                                         name="macc")
                        nc.vector.tensor_add(macc[:], accA[:], accB[:])
                        state["macc"] = macc

                    def norm_colsum(state):
                        # colsum lands in row 0 of an op-ring psum tile so the
                        # pss pair tiles can have the spare PSUM bank
                        macc = state["macc"]
                        psd = psQ.tile([P, SCHUNK], F32, tag="op", bufs=2,
                                       name="psden")
                        nc.tensor.matmul(
                            psd[0:1, :], ones[:], macc[:, 0, :],
                            start=True, stop=False)
                        nc.tensor.matmul(
                            psd[0:1, :], ones[:], macc[:, 1, :],
                            start=False, stop=True)
                        recip = small.tile([1, SCHUNK], F32, tag="recip")
                        nc.vector.reciprocal_approx_fast(recip[:], psd[0:1, :])
                        state["recip"] = recip

                    def norm_bcast(state):
                        rb = small.tile([P, SCHUNK], F32, tag="rb")
                        nc.gpsimd.partition_broadcast(rb[:], state["recip"][:])
                        state["rb"] = rb

                    def norm_mul(state):
                        _, _, pso, ctx_t, h = state["pend"]
                        nc.vector.tensor_mul(ctx_t[:, h, :], pso[:],
                                             state["rb"][:])
                        state["pend"] = None

                    def h_loop(qc, qTs_t, ctx_t, filler, state):
                        """Scores/softmax/attnv for all 4 heads of chunk qc.
                        `filler(i, state)` emits one independent PE matmul per
                        iteration i = h*KT + kt (64 total). The normalization
                        of head h-1 is emitted at head h's kt==6; head 3's is
                        left in state["pend"] for the next chunk's prologue."""
                        for h in range(HLOC):
                            accA = accp.tile([P, 2, SCHUNK], F32, tag="accA")
                            accB = accp.tile([P, 2, SCHUNK], F32, tag="accB")
                            pso = psQ.tile([P, SCHUNK], F32, tag="pso", bufs=2)
                            pairs = []
                            for pi in range(KT // 2):
                                kt0 = 2 * pi
                                # two scores matmuls into one 2-bank psum
                                # pair, then ONE 1024-wide exp (halves the
                                # ACT instruction count)
                                pss = psQ.tile([P, 2, SCHUNK], F32, tag="pss",
                                               bufs=2, name="pss")
                                for j in range(2):
                                    nc.tensor.matmul(
                                        pss[:, j, :],
                                        kT[:, h, (kt0 + j) * P:(kt0 + j + 1) * P],
                                        qTs_t[:, h, :], start=True, stop=True)
                                ep = expp.tile([P, 2, SCHUNK], F32R,
                                               tag="expP", bufs=5, name="ep")
                                pairs.append(ep)
                                nc.scalar.activation(
                                    ep[:], pss[:],
                                    mybir.ActivationFunctionType.Exp,
                                    scale=INV_SQRT_HD)
                                for j in range(2):
                                    kt = kt0 + j
                                    nc.tensor.matmul(
                                        pso[:], vv[:, kt // ST, kt % ST, h, :],
                                        ep[:, j, :],
                                        start=(kt == 0), stop=(kt == KT - 1))
                                    filler(h * KT + kt, state)
                                # Denominator accumulate: DVE takes pairs
                                # {0,2,4,6,7}, Pool {1,3,5} -- the slow Q7
                                # Pool never gates the merge. Head h-1's norm
                                # pieces are spaced so every hop resolves
                                # behind queued PE work (see norm_* above).
                                pend = state["pend"]
                                if pi == 1 and pend is not None:
                                    norm_merge(state)
                                elif pi == 2:
                                    nc.vector.tensor_add(
                                        accA[:], pairs[0][:].bitcast(F32),
                                        pairs[2][:].bitcast(F32))
                                elif pi == 3:
                                    nc.gpsimd.tensor_add(
                                        accB[:], pairs[1][:].bitcast(F32),
                                        pairs[3][:].bitcast(F32))
                                elif pi == 4:
                                    nc.vector.tensor_add(
                                        accA[:], accA[:],
                                        pairs[4][:].bitcast(F32))
                                elif pi == 5:
                                    nc.gpsimd.tensor_add(
                                        accB[:], accB[:],
                                        pairs[5][:].bitcast(F32))
                                elif pi == 6:
                                    nc.vector.tensor_add(
                                        accA[:], accA[:],
                                        pairs[6][:].bitcast(F32))
                                    if pend is not None:
                                        norm_colsum(state)
                                        norm_bcast(state)
                                elif pi == 7:
                                    nc.vector.tensor_add(
                                        accA[:], accA[:],
                                        pairs[7][:].bitcast(F32))
                                    if pend is not None:
                                        norm_mul(state)
                            state["pend"] = (accA, accB, pso, ctx_t, h)

                    # --- QA schedule --------------------------------------
                    xts = [None] * NSC
                    qTs_l = [None] * NSC
                    ctxs = [None] * NSC
                    state = {"pend": None}
                    nofill = lambda i, st: None

                    xts[0] = load_x(0)
                    xts[1] = load_x(1)
                    # q-projection block for chunk 0 (PE filler: none needed,
                    # the PE is saturated by the block itself)
                    qTs_l[0] = qts.tile([P, HLOC, SCHUNK], BF16, tag="qT", name="qT0")
                    st0 = {}
                    for i in range(MT * DT):
                        qproj_leg(qTs_l[0], xts[0], i, st0)

                    for qc in range(NSC):
                        ctxs[qc] = ctxp.tile([P, HLOC, SCHUNK], BF16, tag="ctx", name="ctx")
                        if qc == 0:
                            # interleave chunk 1's qproj legs mt=0..2
                            xts[2] = load_x(2)
                            qTs_l[1] = qts.tile([P, HLOC, SCHUNK], BF16,
                                                tag="qT", name="qT1")
                            filler = (lambda i, st:
                                      qproj_leg(qTs_l[1], xts[1], i, st)
                                      if i < 48 else None)
                        else:
                            # prologue: finish/run this chunk's qproj, with
                            # the previous chunk's head-3 norm pieces spaced
                            # through it (psden allocated after the second
                            # chain so the op-ring phase stays clean)
                            if qc == 1:
                                xts[3] = load_x(3)
                                for i in range(48, 64):
                                    qproj_leg(qTs_l[1], xts[1], i, state)
                                    if i == 51:
                                        norm_merge(state)
                                    elif i == 54:
                                        norm_colsum(state)
                                    elif i == 58:
                                        norm_bcast(state)
                                    elif i == 61:
                                        norm_mul(state)
                            else:
                                qTs_l[qc] = qts.tile([P, HLOC, SCHUNK], BF16,
                                                     tag="qT", name="qTn")
                                for i in range(MT * DT):
                                    qproj_leg(qTs_l[qc], xts[qc], i, state)
                                    if i == 3:
                                        norm_merge(state)
                                    elif i == DT + 1:
                                        norm_colsum(state)
                                    elif i == DT + 5:
                                        norm_bcast(state)
                                    elif i == DT + 9:
                                        norm_mul(state)
                            pctx = ctxs[qc - 1]
                            pqc = qc - 1
                            filler = (lambda i, st, c=pctx, q=pqc:
                                      oproj_leg(c, q, i, st))
                        h_loop(qc, qTs_l[qc], ctxs[qc], filler, state)

                    # tail: o_proj for the last chunk. Chain 0's jt=0..2
                    # legs need only ctx[0..2], so they cover the last head's
                    # norm pieces; its jt=3 leg follows the multiply.
                    norm_merge(state)
                    st_a = {}
                    for leg in range(MT - 1):
                        oproj_leg(ctxs[NSC - 1], NSC - 1, leg, st_a)
                    norm_colsum(state)
                    norm_bcast(state)
                    norm_mul(state)
                    oproj_leg(ctxs[NSC - 1], NSC - 1, MT - 1, st_a)
                    for i in range(MT, HLOC * KT):
                        oproj_leg(ctxs[NSC - 1], NSC - 1, i, state)

    nc.finalize()
    return nc

_build = build


def _bf16(a):
    return np.ascontiguousarray(a, dtype=np.float32).astype(ml_dtypes.bfloat16)


def kernel(hidden_states, wq, wk, wv, wo):
    global last_run
    if "nc" not in _cache:
        _cache["nc"] = build()
    nc = _cache["nc"]

    hidden_states = np.asarray(hidden_states, dtype=np.float32)
    wq = np.asarray(wq, dtype=np.float32)
    wk = np.asarray(wk, dtype=np.float32)
    wv = np.asarray(wv, dtype=np.float32)
    wo = np.asarray(wo, dtype=np.float32)

    xT = [_bf16(hidden_states[b].T) for b in range(B)]
    in_maps = []
    for c in range(NCORES):
        b, g = divmod(c, G)
        sl = slice(g * DG, (g + 1) * DG)
        in_maps.append({
            "xT": xT[b],
            "wqT": _bf16(wq[sl, :].T),
            "wkT": _bf16(wk[sl, :].T),
            "wvT": _bf16(wv[sl, :].T),
            "woT": _bf16(wo[:, sl].T),
        })

    trace = os.environ.get("BASSKERNEL_TRACE", "0") == "1"
    last_run = run_bass_kernel_spmd(
        nc, in_maps, core_ids=list(range(NCORES)), trace=trace)

    out = np.empty((B, S, D), dtype=np.float32)
    for b in range(B):
        acc = None
        for g in range(G):
            part = last_run.results[b * G + g]["out"]
            acc = part.copy() if acc is None else acc + part
        out[b] = acc
    return out


# revision 17
# speedup vs baseline: 1.1217x; 1.0007x over previous
"""Multi-head attention (B=2, S=2048, D=2048, H=16) on 8 TRN2 NeuronCores.

Sharding: data-parallel over batch (2) x Megatron tensor-parallel over heads
(4 groups of 4 heads). Core c = 4*b + g handles batch b, heads [4g, 4g+4).
Each core computes q/k/v projections for its head slice, attention over its
4 heads, and a partial o_proj contribution; the host sums the 4 partials per
batch (the unshard step of Megatron TP) and stacks the 2 batches.

Precision: projections and o_proj run in bf16 (x, wq/wk/wv, kT, qT, ctx, wo);
attention-value and softmax stay in f32r/f32 (vv, expP, denominators). All
matmul accumulation is fp32 in PSUM. Measured max-rel-err ~6e-3 vs the fp32
reference (harness gate 2e-2).

Schedule (single pass, everything SBUF-resident, no DRAM spills):
  Prefix: per 512-seq chunk {k-projection -> kT, v-projection -> vv}.
  QA (per 512-query chunk): a per-head kt-loop {scores matmul, exp (ACT),
  pairwise denominator accumulate (DVE/Pool alternating), attn@v accumulate}
  with one independent matmul interleaved per iteration so the in-order
  tensor-engine queue never drains: chunk 0 interleaves chunk 1's
  q-projection legs, chunks 1-3 interleave the PREVIOUS chunk's o_proj legs.
  Per-head normalization (colsum ones-matmul on fp32r-bitcast accumulators,
  reciprocal_approx_fast, partition_broadcast, multiply) is software-
  pipelined one head late so its cross-engine dependency chain resolves
  behind ~4us of queued PE work instead of stalling the PE.

Why: the PE is the roofline engine (~331us of unavoidable matmul rows per
core at 2.4 GHz) and the HAM clock gate halves its clock after ~3.4us of
idle, so the whole design optimizes for an unbroken PE instruction stream.
"""

import contextlib
import math
import os

import numpy as np
import ml_dtypes

import concourse.mybir as mybir
import concourse.tile as tile
from concourse import bacc
from concourse.bass_utils import run_bass_kernel_spmd

F32 = mybir.dt.float32
F32R = mybir.dt.float32r
BF16 = mybir.dt.bfloat16

B, S, D = 2, 2048, 2048
H = 16
HD = 128
G = 4                    # tensor-parallel groups
HLOC = H // G            # heads per core = 4
DG = HLOC * HD           # per-core projection width = 512
P = 128
NCORES = 8

SCHUNK = 512             # seq chunk
NSC = S // SCHUNK        # 4 chunks
ST = SCHUNK // P         # 4 s-tiles per chunk
DT = D // P              # 16 contraction tiles
MT = DG // P             # 4 output tiles (== HLOC)
KT = S // P              # 16 key tiles
IC = D // SCHUNK         # 4 o_proj output column blocks
INV_SQRT_HD = 1.0 / math.sqrt(HD)

_cache = {}
last_run = None  # BassKernelResults of the most recent execution (for test.py)


def build(loop_reps=None):
    nc = bacc.Bacc(None, target_bir_lowering=False)

    xT_dr = nc.dram_tensor("xT", [D, S], BF16, kind="ExternalInput")
    wqT_dr = nc.dram_tensor("wqT", [D, DG], BF16, kind="ExternalInput")
    wkT_dr = nc.dram_tensor("wkT", [D, DG], BF16, kind="ExternalInput")
    wvT_dr = nc.dram_tensor("wvT", [D, DG], BF16, kind="ExternalInput")
    woT_dr = nc.dram_tensor("woT", [DG, D], BF16, kind="ExternalInput")
    out_d = nc.dram_tensor("out", [S, D], F32, kind="ExternalOutput")

    xT_view = xT_dr.rearrange("(o p) s -> p o s", p=P)
    wkT_v = wkT_dr.rearrange("(o p) m -> p o m", p=P)
    wqT_v = wqT_dr.rearrange("(o p) m -> p o m", p=P)
    wvT_v = wvT_dr.rearrange("(o p) m -> p o m", p=P)
    woT_v = woT_dr.rearrange("(o p) i -> p o i", p=P)

    with tile.TileContext(nc) as tc:
        loop_cm = tc.For_i(0, loop_reps, 1) if loop_reps else contextlib.nullcontext()
        with loop_cm:
            with (
                tc.tile_pool(name="persist", bufs=1) as pers,
                tc.tile_pool(name="xring", bufs=3) as xring,
            ):
                kT = pers.tile([P, HLOC, S], BF16, tag="kT")
                vv = pers.tile([P, NSC, ST, HLOC, HD], F32R, tag="vv")
                wq = pers.tile([P, DT, DG], BF16, tag="wq")
                wo = pers.tile([P, MT, D], BF16, tag="wo")
                ones = pers.tile([P, 1], F32R, tag="ones")
                ones_f = pers.tile([P, 1], F32, tag="ones_f")
                nc.vector.memset(ones_f[:], 1.0)
                nc.vector.tensor_copy(ones[:], ones_f[:])

                def load_x(sc):
                    xt = xring.tile([P, DT, SCHUNK], BF16, tag="x")
                    c0 = sc * SCHUNK
                    for d0 in range(0, DT, 4):
                        nc.sync.dma_start(
                            xt[:, d0:d0 + 4], xT_view[:, d0:d0 + 4, c0:c0 + SCHUNK])
                    return xt

                # ---------- prefix: k/v projections, SBUF-resident ----------
                with (
                    tc.tile_pool(name="wkv", bufs=1) as wkvp,
                    tc.tile_pool(name="psumP", bufs=1, space="PSUM") as psP,
                ):
                    wk = wkvp.tile([P, DT, DG], BF16, tag="wk")
                    wv = wkvp.tile([P, DT, DG], BF16, tag="wv")
                    # DMA order matters: the first k-matmul needs wk + x(0),
                    # so x(0) goes right after wk, ahead of wv/wq/wo
                    for d0 in range(0, DT, 4):
                        nc.sync.dma_start(wk[:, d0:d0 + 4], wkT_v[:, d0:d0 + 4])
                    xt0 = load_x(0)
                    for d0 in range(0, DT, 4):
                        nc.sync.dma_start(wv[:, d0:d0 + 4], wvT_v[:, d0:d0 + 4])
                    for d0 in range(0, DT, 4):
                        nc.sync.dma_start(wq[:, d0:d0 + 4], wqT_v[:, d0:d0 + 4])
                    for j0 in range(MT):
                        nc.sync.dma_start(wo[:, j0:j0 + 1], woT_v[:, j0:j0 + 1])

                    for sc in range(NSC):
                        xt = xt0 if sc == 0 else load_x(sc)
                        c0 = sc * SCHUNK
                        for mt in range(MT):
                            ps = psP.tile([P, SCHUNK], F32, tag="kps", bufs=3)
                            for dt in range(DT):
                                nc.tensor.matmul(
                                    ps[:], wk[:, dt, mt * P:(mt + 1) * P],
                                    xt[:, dt, :],
                                    start=(dt == 0), stop=(dt == DT - 1))
                            nc.vector.tensor_copy(kT[:, mt, c0:c0 + SCHUNK], ps[:])
                        for st in range(ST):
                            ps = psP.tile([P, DG], F32, tag="vps", bufs=3)
                            for dt in range(DT):
                                nc.tensor.matmul(
                                    ps[:], xt[:, dt, st * P:(st + 1) * P],
                                    wv[:, dt, :],
                                    start=(dt == 0), stop=(dt == DT - 1))
                            nc.vector.tensor_copy(vv[:, sc, st], ps[:])

                # ---------- QA: per query chunk ----------
                with (
                    tc.tile_pool(name="qts", bufs=2) as qts,
                    tc.tile_pool(name="ctxp", bufs=2) as ctxp,
                    tc.tile_pool(name="expp", bufs=4) as expp,
                    tc.tile_pool(name="accp", bufs=2) as accp,
                    tc.tile_pool(name="small", bufs=2) as small,
                    tc.tile_pool(name="psumQ", bufs=1, space="PSUM") as psQ,
                ):
                    # --- emit helpers -------------------------------------
                    def qproj_leg(qTs_t, xt, i, state):
                        """q-projection leg i (0..63): mt = i//16, dt = i%16."""
                        mt, dt = divmod(i, DT)
                        if dt == 0:
                            state["qp"] = psQ.tile([P, SCHUNK], F32, tag="op",
                                                   bufs=2, name="qpps")
                        ps = state["qp"]
                        nc.tensor.matmul(
                            ps[:], wq[:, dt, mt * P:(mt + 1) * P], xt[:, dt, :],
                            start=(dt == 0), stop=(dt == DT - 1))
                        if dt == DT - 1:
                            nc.scalar.copy(qTs_t[:, mt, :], ps[:])

                    def oproj_leg(ctx_t, pqc, i, state):
                        """o_proj leg i (0..63) of chunk pqc: chain=i//4."""
                        chain, leg = divmod(i, MT)
                        st, ic = divmod(chain, IC)
                        if leg == 0:
                            state["op"] = psQ.tile([P, SCHUNK], F32, tag="op",
                                                   bufs=2, name="opps")
                        ps = state["op"]
                        nc.tensor.matmul(
                            ps[:], ctx_t[:, leg, st * P:(st + 1) * P],
                            wo[:, leg, ic * SCHUNK:(ic + 1) * SCHUNK],
                            start=(leg == 0), stop=(leg == MT - 1))
                        if leg == MT - 1:
                            ob = small.tile([P, SCHUNK], F32, tag="ob", bufs=3)
                            if chain % 4 == 0:
                                nc.scalar.copy(ob[:], ps[:])
                            else:
                                nc.vector.tensor_copy(ob[:], ps[:])
                            r0 = pqc * SCHUNK + st * P
                            nc.sync.dma_start(
                                out_d[r0:r0 + P, ic * SCHUNK:(ic + 1) * SCHUNK],
                                ob[:])

                    # The per-head normalization is emitted in PIECES so
                    # each cross-engine hop (DVE merge -> PE colsum -> DVE
                    # recip -> Pool bcast -> DVE mul) resolves behind queued
                    # PE work, and no productive engine op ever sits behind a
                    # chain wait in its in-order queue.
                    def norm_merge(state):
                        accA, accB, _, _, _ = state["pend"]
                        # cast-on-write merge: the BIR verifier requires f32r
                        # matmul operands to be f32r-rounded by their producer
                        macc = accp.tile([P, 2, SCHUNK], F32R, tag="macc",
                                         name="macc")
                        nc.vector.tensor_add(macc[:], accA[:], accB[:])
                        state["macc"] = macc

                    def norm_colsum(state):
                        # colsum lands in row 0 of an op-ring psum tile so the
                        # pss pair tiles can have the spare PSUM bank
                        macc = state["macc"]
                        psd = psQ.tile([P, SCHUNK], F32, tag="op", bufs=2,
                                       name="psden")
                        nc.tensor.matmul(
                            psd[0:1, :], ones[:], macc[:, 0, :],
                            start=True, stop=False)
                        nc.tensor.matmul(
                            psd[0:1, :], ones[:], macc[:, 1, :],
                            start=False, stop=True)
                        recip = small.tile([1, SCHUNK], F32, tag="recip")
                        nc.vector.reciprocal_approx_fast(recip[:], psd[0:1, :])
                        state["recip"] = recip

                    def norm_bcast(state):
                        rb = small.tile([P, SCHUNK], F32, tag="rb")
                        nc.gpsimd.partition_broadcast(rb[:], state["recip"][:])
                        state["rb"] = rb

                    def norm_mul(state):
                        _, _, pso, ctx_t, h = state["pend"]
                        nc.vector.tensor_mul(ctx_t[:, h, :], pso[:],
                                             state["rb"][:])
                        state["pend"] = None

                    def h_loop(qc, qTs_t, ctx_t, filler, state):
                        """Scores/softmax/attnv for all 4 heads of chunk qc.
                        `filler(i, state)` emits one independent PE matmul per
                        iteration i = h*KT + kt (64 total). The normalization
                        of head h-1 is emitted at head h's kt==6; head 3's is
                        left in state["pend"] for the next chunk's prologue."""
                        for h in range(HLOC):
                            accA = accp.tile([P, 2, SCHUNK], F32, tag="accA")
                            accB = accp.tile([P, 2, SCHUNK], F32, tag="accB")
                            pso = psQ.tile([P, SCHUNK], F32, tag="pso", bufs=2)
                            pairs = []
                            for pi in range(KT // 2):
                                kt0 = 2 * pi
                                # two scores matmuls into one 2-bank psum
                                # pair, then ONE 1024-wide exp (halves the
                                # ACT instruction count)
                                pss = psQ.tile([P, 2, SCHUNK], F32, tag="pss",
                                               bufs=2, name="pss")
                                for j in range(2):
                                    nc.tensor.matmul(
                                        pss[:, j, :],
                                        kT[:, h, (kt0 + j) * P:(kt0 + j + 1) * P],
                                        qTs_t[:, h, :], start=True, stop=True)
                                ep = expp.tile([P, 2, SCHUNK], F32R,
                                               tag="expP", bufs=6, name="ep")
                                pairs.append(ep)
                                nc.scalar.activation(
                                    ep[:], pss[:],
                                    mybir.ActivationFunctionType.Exp,
                                    scale=INV_SQRT_HD)
                                for j in range(2):
                                    kt = kt0 + j
                                    nc.tensor.matmul(
                                        pso[:], vv[:, kt // ST, kt % ST, h, :],
                                        ep[:, j, :],
                                        start=(kt == 0), stop=(kt == KT - 1))
                                    filler(h * KT + kt, state)
                                # Denominator accumulate: DVE takes pairs
                                # {0,2,4,6,7}, Pool {1,3,5} -- the slow Q7
                                # Pool never gates the merge. Head h-1's norm
                                # pieces are spaced so every hop resolves
                                # behind queued PE work (see norm_* above).
                                pend = state["pend"]
                                if pi == 1 and pend is not None:
                                    norm_merge(state)
                                elif pi == 2:
                                    nc.vector.tensor_add(
                                        accA[:], pairs[0][:].bitcast(F32),
                                        pairs[2][:].bitcast(F32))
                                elif pi == 3:
                                    nc.gpsimd.tensor_add(
                                        accB[:], pairs[1][:].bitcast(F32),
                                        pairs[3][:].bitcast(F32))
                                elif pi == 4:
                                    nc.vector.tensor_add(
                                        accA[:], accA[:],
                                        pairs[4][:].bitcast(F32))
                                elif pi == 5:
                                    nc.gpsimd.tensor_add(
                                        accB[:], accB[:],
                                        pairs[5][:].bitcast(F32))
                                elif pi == 6:
                                    nc.vector.tensor_add(
                                        accA[:], accA[:],
                                        pairs[6][:].bitcast(F32))
                                    if pend is not None:
                                        norm_colsum(state)
                                        norm_bcast(state)
                                elif pi == 7:
                                    nc.vector.tensor_add(
                                        accA[:], accA[:],
                                        pairs[7][:].bitcast(F32))
                                    if pend is not None:
                                        norm_mul(state)
                            state["pend"] = (accA, accB, pso, ctx_t, h)

                    # --- QA schedule --------------------------------------
                    xts = [None] * NSC
                    qTs_l = [None] * NSC
                    ctxs = [None] * NSC
                    state = {"pend": None}
                    nofill = lambda i, st: None

                    xts[0] = load_x(0)
                    xts[1] = load_x(1)
                    # q-projection block for chunk 0 (PE filler: none needed,
                    # the PE is saturated by the block itself)
                    qTs_l[0] = qts.tile([P, HLOC, SCHUNK], BF16, tag="qT", name="qT0")
                    st0 = {}
                    for i in range(MT * DT):
                        qproj_leg(qTs_l[0], xts[0], i, st0)

                    for qc in range(NSC):
                        ctxs[qc] = ctxp.tile([P, HLOC, SCHUNK], BF16, tag="ctx", name="ctx")
                        if qc == 0:
                            # interleave chunk 1's qproj legs mt=0..2
                            xts[2] = load_x(2)
                            qTs_l[1] = qts.tile([P, HLOC, SCHUNK], BF16,
                                                tag="qT", name="qT1")
                            filler = (lambda i, st:
                                      qproj_leg(qTs_l[1], xts[1], i, st)
                                      if i < 48 else None)
                        else:
                            # prologue: finish/run this chunk's qproj, with
                            # the previous chunk's head-3 norm pieces spaced
                            # through it (psden allocated after the second
                            # chain so the op-ring phase stays clean)
                            if qc == 1:
                                xts[3] = load_x(3)
                                for i in range(48, 64):
                                    qproj_leg(qTs_l[1], xts[1], i, state)
                                    if i == 51:
                                        norm_merge(state)
                                    elif i == 54:
                                        norm_colsum(state)
                                    elif i == 58:
                                        norm_bcast(state)
                                    elif i == 61:
                                        norm_mul(state)
                            else:
                                qTs_l[qc] = qts.tile([P, HLOC, SCHUNK], BF16,
                                                     tag="qT", name="qTn")
                                for i in range(MT * DT):
                                    qproj_leg(qTs_l[qc], xts[qc], i, state)
                                    if i == 3:
                                        norm_merge(state)
                                    elif i == DT + 1:
                                        norm_colsum(state)
                                    elif i == DT + 5:
                                        norm_bcast(state)
                                    elif i == DT + 9:
                                        norm_mul(state)
                            pctx = ctxs[qc - 1]
                            pqc = qc - 1
                            filler = (lambda i, st, c=pctx, q=pqc:
                                      oproj_leg(c, q, i, st))
                        h_loop(qc, qTs_l[qc], ctxs[qc], filler, state)

                    # tail: o_proj for the last chunk. Chain 0's jt=0..2
                    # legs need only ctx[0..2], so they cover the last head's
                    # norm pieces; its jt=3 leg follows the multiply.
                    norm_merge(state)
                    st_a = {}
                    for leg in range(MT - 1):
                        oproj_leg(ctxs[NSC - 1], NSC - 1, leg, st_a)
                    norm_colsum(state)
                    norm_bcast(state)
                    norm_mul(state)
                    oproj_leg(ctxs[NSC - 1], NSC - 1, MT - 1, st_a)
                    for i in range(MT, HLOC * KT):
                        oproj_leg(ctxs[NSC - 1], NSC - 1, i, state)

    nc.finalize()
    return nc

_build = build


def _bf16(a):
    return np.ascontiguousarray(a, dtype=np.float32).astype(ml_dtypes.bfloat16)


def kernel(hidden_states, wq, wk, wv, wo):
    global last_run
    if "nc" not in _cache:
        _cache["nc"] = build()
    nc = _cache["nc"]

    hidden_states = np.asarray(hidden_states, dtype=np.float32)
    wq = np.asarray(wq, dtype=np.float32)
    wk = np.asarray(wk, dtype=np.float32)
    wv = np.asarray(wv, dtype=np.float32)
    wo = np.asarray(wo, dtype=np.float32)

    xT = [_bf16(hidden_states[b].T) for b in range(B)]
    in_maps = []
    for c in range(NCORES):
        b, g = divmod(c, G)
        sl = slice(g * DG, (g + 1) * DG)
        in_maps.append({
            "xT": xT[b],
            "wqT": _bf16(wq[sl, :].T),
            "wkT": _bf16(wk[sl, :].T),
            "wvT": _bf16(wv[sl, :].T),
            "woT": _bf16(wo[:, sl].T),
        })

    trace = os.environ.get("BASSKERNEL_TRACE", "0") == "1"
    last_run = run_bass_kernel_spmd(
        nc, in_maps, core_ids=list(range(NCORES)), trace=trace)

    out = np.empty((B, S, D), dtype=np.float32)
    for b in range(B):
        acc = None
        for g in range(G):
            part = last_run.results[b * G + g]["out"]
            acc = part.copy() if acc is None else acc + part
        out[b] = acc
    return out


# revision 18
# speedup vs baseline: 1.1372x; 1.0138x over previous
"""Multi-head attention (B=2, S=2048, D=2048, H=16) on 8 TRN2 NeuronCores.

Sharding: data-parallel over batch (2) x Megatron tensor-parallel over heads
(4 groups of 4 heads). Core c = 4*b + g handles batch b, heads [4g, 4g+4).
Each core computes q/k/v projections for its head slice, attention over its
4 heads, and a partial o_proj contribution; the host sums the 4 partials per
batch (the unshard step of Megatron TP) and stacks the 2 batches.

Precision: projections and o_proj run in bf16 (x, wq/wk/wv, kT, qT, ctx, wo);
attention-value and softmax stay in f32r/f32 (vv, expP, denominators). All
matmul accumulation is fp32 in PSUM. Measured max-rel-err ~6e-3 vs the fp32
reference (harness gate 2e-2).

Schedule (single pass, everything SBUF-resident, no DRAM spills):
  Prefix: per 512-seq chunk {k-projection -> kT, v-projection -> vv}.
  QA (per 512-query chunk): a per-head kt-loop {scores matmul, exp (ACT),
  pairwise denominator accumulate (DVE/Pool alternating), attn@v accumulate}
  with one independent matmul interleaved per iteration so the in-order
  tensor-engine queue never drains: chunk 0 interleaves chunk 1's
  q-projection legs, chunks 1-3 interleave the PREVIOUS chunk's o_proj legs.
  Per-head normalization (colsum ones-matmul on fp32r-bitcast accumulators,
  reciprocal_approx_fast, partition_broadcast, multiply) is software-
  pipelined one head late so its cross-engine dependency chain resolves
  behind ~4us of queued PE work instead of stalling the PE.

Why: the PE is the roofline engine (~331us of unavoidable matmul rows per
core at 2.4 GHz) and the HAM clock gate halves its clock after ~3.4us of
idle, so the whole design optimizes for an unbroken PE instruction stream.
"""

import contextlib
import math
import os

import numpy as np
import ml_dtypes

import concourse.mybir as mybir
import concourse.tile as tile
from concourse import bacc
from concourse.bass_utils import run_bass_kernel_spmd

F32 = mybir.dt.float32
F32R = mybir.dt.float32r
BF16 = mybir.dt.bfloat16

B, S, D = 2, 2048, 2048
H = 16
HD = 128
G = 4                    # tensor-parallel groups
HLOC = H // G            # heads per core = 4
DG = HLOC * HD           # per-core projection width = 512
P = 128
NCORES = 8

SCHUNK = 512             # seq chunk
NSC = S // SCHUNK        # 4 chunks
ST = SCHUNK // P         # 4 s-tiles per chunk
DT = D // P              # 16 contraction tiles
MT = DG // P             # 4 output tiles (== HLOC)
KT = S // P              # 16 key tiles
IC = D // SCHUNK         # 4 o_proj output column blocks
INV_SQRT_HD = 1.0 / math.sqrt(HD)

_cache = {}
last_run = None  # BassKernelResults of the most recent execution (for test.py)


def build(loop_reps=None):
    nc = bacc.Bacc(None, target_bir_lowering=False)

    xT_dr = nc.dram_tensor("xT", [D, S], BF16, kind="ExternalInput")
    wqT_dr = nc.dram_tensor("wqT", [D, DG], BF16, kind="ExternalInput")
    wkT_dr = nc.dram_tensor("wkT", [D, DG], BF16, kind="ExternalInput")
    wvT_dr = nc.dram_tensor("wvT", [D, DG], BF16, kind="ExternalInput")
    woT_dr = nc.dram_tensor("woT", [DG, D], BF16, kind="ExternalInput")
    out_d = nc.dram_tensor("out", [S, D], F32, kind="ExternalOutput")

    xT_view = xT_dr.rearrange("(o p) s -> p o s", p=P)
    wkT_v = wkT_dr.rearrange("(o p) m -> p o m", p=P)
    wqT_v = wqT_dr.rearrange("(o p) m -> p o m", p=P)
    wvT_v = wvT_dr.rearrange("(o p) m -> p o m", p=P)
    woT_v = woT_dr.rearrange("(o p) i -> p o i", p=P)

    with tile.TileContext(nc) as tc:
        loop_cm = tc.For_i(0, loop_reps, 1) if loop_reps else contextlib.nullcontext()
        with loop_cm:
            with (
                tc.tile_pool(name="persist", bufs=1) as pers,
                tc.tile_pool(name="xring", bufs=3) as xring,
            ):
                kT = pers.tile([P, HLOC, S], BF16, tag="kT")
                vv = pers.tile([P, NSC, ST, HLOC, HD], F32R, tag="vv")
                wq = pers.tile([P, DT, DG], BF16, tag="wq")
                wo = pers.tile([P, MT, D], BF16, tag="wo")
                ones = pers.tile([P, 1], F32R, tag="ones")
                ones_f = pers.tile([P, 1], F32, tag="ones_f")
                nc.vector.memset(ones_f[:], 1.0)
                nc.vector.tensor_copy(ones[:], ones_f[:])

                def load_x(sc):
                    xt = xring.tile([P, DT, SCHUNK], BF16, tag="x")
                    c0 = sc * SCHUNK
                    for d0 in range(0, DT, 4):
                        nc.sync.dma_start(
                            xt[:, d0:d0 + 4], xT_view[:, d0:d0 + 4, c0:c0 + SCHUNK])
                    return xt

                # ---------- prefix: k/v projections, SBUF-resident ----------
                with (
                    tc.tile_pool(name="wkv", bufs=1) as wkvp,
                    tc.tile_pool(name="psumP", bufs=1, space="PSUM") as psP,
                ):
                    wk = wkvp.tile([P, DT, DG], BF16, tag="wk")
                    wv = wkvp.tile([P, DT, DG], BF16, tag="wv")
                    # DMA order matters: the first k-matmul needs wk + x(0),
                    # so x(0) goes right after wk, ahead of wv/wq/wo
                    for d0 in range(0, DT, 4):
                        nc.sync.dma_start(wk[:, d0:d0 + 4], wkT_v[:, d0:d0 + 4])
                    xt0 = load_x(0)
                    for d0 in range(0, DT, 4):
                        nc.sync.dma_start(wv[:, d0:d0 + 4], wvT_v[:, d0:d0 + 4])
                    for d0 in range(0, DT, 4):
                        nc.sync.dma_start(wq[:, d0:d0 + 4], wqT_v[:, d0:d0 + 4])
                    for j0 in range(MT):
                        nc.sync.dma_start(wo[:, j0:j0 + 1], woT_v[:, j0:j0 + 1])

                    for sc in range(NSC):
                        xt = xt0 if sc == 0 else load_x(sc)
                        c0 = sc * SCHUNK
                        for mt in range(MT):
                            ps = psP.tile([P, SCHUNK], F32, tag="kps", bufs=3)
                            for dt in range(DT):
                                nc.tensor.matmul(
                                    ps[:], wk[:, dt, mt * P:(mt + 1) * P],
                                    xt[:, dt, :],
                                    start=(dt == 0), stop=(dt == DT - 1))
                            nc.vector.tensor_copy(kT[:, mt, c0:c0 + SCHUNK], ps[:])
                        for st in range(ST):
                            ps = psP.tile([P, DG], F32, tag="vps", bufs=3)
                            for dt in range(DT):
                                nc.tensor.matmul(
                                    ps[:], xt[:, dt, st * P:(st + 1) * P],
                                    wv[:, dt, :],
                                    start=(dt == 0), stop=(dt == DT - 1))
                            nc.vector.tensor_copy(vv[:, sc, st], ps[:])

                # ---------- QA: per query chunk ----------
                with (
                    tc.tile_pool(name="qts", bufs=2) as qts,
                    tc.tile_pool(name="ctxp", bufs=2) as ctxp,
                    tc.tile_pool(name="expp", bufs=4) as expp,
                    tc.tile_pool(name="accp", bufs=2) as accp,
                    tc.tile_pool(name="small", bufs=2) as small,
                    tc.tile_pool(name="psumQ", bufs=1, space="PSUM") as psQ,
                ):
                    # --- emit helpers -------------------------------------
                    def qproj_leg(qTs_t, xt, i, state):
                        """q-projection leg i (0..63): mt = i//16, dt = i%16."""
                        mt, dt = divmod(i, DT)
                        if dt == 0:
                            state["qp"] = psQ.tile([P, SCHUNK], F32, tag="op",
                                                   bufs=2, name="qpps")
                        ps = state["qp"]
                        nc.tensor.matmul(
                            ps[:], wq[:, dt, mt * P:(mt + 1) * P], xt[:, dt, :],
                            start=(dt == 0), stop=(dt == DT - 1))
                        if dt == DT - 1:
                            nc.scalar.copy(qTs_t[:, mt, :], ps[:])

                    def oproj_leg(ctx_t, pqc, i, state):
                        """o_proj leg i (0..63) of chunk pqc: chain=i//4."""
                        chain, leg = divmod(i, MT)
                        st, ic = divmod(chain, IC)
                        if leg == 0:
                            state["op"] = psQ.tile([P, SCHUNK], F32, tag="op",
                                                   bufs=2, name="opps")
                        ps = state["op"]
                        nc.tensor.matmul(
                            ps[:], ctx_t[:, leg, st * P:(st + 1) * P],
                            wo[:, leg, ic * SCHUNK:(ic + 1) * SCHUNK],
                            start=(leg == 0), stop=(leg == MT - 1))
                        if leg == MT - 1:
                            ob = small.tile([P, SCHUNK], F32, tag="ob", bufs=3)
                            if chain % 4 == 0:
                                nc.scalar.copy(ob[:], ps[:])
                            else:
                                nc.vector.tensor_copy(ob[:], ps[:])
                            r0 = pqc * SCHUNK + st * P
                            nc.sync.dma_start(
                                out_d[r0:r0 + P, ic * SCHUNK:(ic + 1) * SCHUNK],
                                ob[:])

                    # The per-head normalization is emitted in PIECES so
                    # each cross-engine hop (DVE merge -> PE colsum -> DVE
                    # recip -> Pool bcast -> DVE mul) resolves behind queued
                    # PE work, and no productive engine op ever sits behind a
                    # chain wait in its in-order queue.
                    def norm_merge(state):
                        # cast accB to f32r (verifier: f32r matmul operands
                        # must be f32r-rounded by their producer). accA is
                        # already accumulated in f32r, so the colsum chain no
                        # longer waits on any mid-head DVE backlog.
                        _, accB, _, _, _ = state["pend"]
                        macc = accp.tile([P, 2, SCHUNK], F32R, tag="macc",
                                         name="macc")
                        nc.vector.tensor_copy(macc[:], accB[:])
                        state["macc"] = macc

                    def norm_colsum(state):
                        # colsum lands in row 0 of an op-ring psum tile so the
                        # pss pair tiles can have the spare PSUM bank
                        macc = state["macc"]
                        accA = state["pend"][0]
                        psd = psQ.tile([P, SCHUNK], F32, tag="op", bufs=2,
                                       name="psden")
                        nc.tensor.matmul(
                            psd[0:1, :], ones[:], macc[:, 0, :],
                            start=True, stop=False)
                        nc.tensor.matmul(
                            psd[0:1, :], ones[:], macc[:, 1, :],
                            start=False, stop=False)
                        nc.tensor.matmul(
                            psd[0:1, :], ones[:], accA[:, 0, :],
                            start=False, stop=False)
                        nc.tensor.matmul(
                            psd[0:1, :], ones[:], accA[:, 1, :],
                            start=False, stop=True)
                        recip = small.tile([1, SCHUNK], F32, tag="recip")
                        nc.vector.reciprocal_approx_fast(recip[:], psd[0:1, :])
                        state["recip"] = recip

                    def norm_bcast(state):
                        rb = small.tile([P, SCHUNK], F32, tag="rb")
                        nc.gpsimd.partition_broadcast(rb[:], state["recip"][:])
                        state["rb"] = rb

                    def norm_mul(state):
                        _, _, pso, ctx_t, h = state["pend"]
                        nc.vector.tensor_mul(ctx_t[:, h, :], pso[:],
                                             state["rb"][:])
                        state["pend"] = None

                    def h_loop(qc, qTs_t, ctx_t, filler, state):
                        """Scores/softmax/attnv for all 4 heads of chunk qc.
                        `filler(i, state)` emits one independent PE matmul per
                        iteration i = h*KT + kt (64 total). The normalization
                        of head h-1 is emitted at head h's kt==6; head 3's is
                        left in state["pend"] for the next chunk's prologue."""
                        for h in range(HLOC):
                            accA = accp.tile([P, 2, SCHUNK], F32R, tag="accA")
                            accB = accp.tile([P, 2, SCHUNK], F32, tag="accB")
                            pso = psQ.tile([P, SCHUNK], F32, tag="pso", bufs=2)
                            pairs = []
                            for pi in range(KT // 2):
                                kt0 = 2 * pi
                                # two scores matmuls into one 2-bank psum
                                # pair, then ONE 1024-wide exp (halves the
                                # ACT instruction count)
                                pss = psQ.tile([P, 2, SCHUNK], F32, tag="pss",
                                               bufs=2, name="pss")
                                for j in range(2):
                                    nc.tensor.matmul(
                                        pss[:, j, :],
                                        kT[:, h, (kt0 + j) * P:(kt0 + j + 1) * P],
                                        qTs_t[:, h, :], start=True, stop=True)
                                ep = expp.tile([P, 2, SCHUNK], F32R,
                                               tag="expP", bufs=6, name="ep")
                                pairs.append(ep)
                                nc.scalar.activation(
                                    ep[:], pss[:],
                                    mybir.ActivationFunctionType.Exp,
                                    scale=INV_SQRT_HD)
                                for j in range(2):
                                    kt = kt0 + j
                                    nc.tensor.matmul(
                                        pso[:], vv[:, kt // ST, kt % ST, h, :],
                                        ep[:, j, :],
                                        start=(kt == 0), stop=(kt == KT - 1))
                                    filler(h * KT + kt, state)
                                # Denominator accumulate: DVE takes pairs
                                # {0,2,4,6,7}, Pool {1,3,5} -- the slow Q7
                                # Pool never gates the merge. Head h-1's norm
                                # pieces are spaced so every hop resolves
                                # behind queued PE work (see norm_* above).
                                pend = state["pend"]
                                if pi == 1 and pend is not None:
                                    norm_merge(state)
                                elif pi == 2:
                                    nc.vector.tensor_add(
                                        accA[:], pairs[0][:].bitcast(F32),
                                        pairs[2][:].bitcast(F32))
                                elif pi == 3:
                                    nc.gpsimd.tensor_add(
                                        accB[:], pairs[1][:].bitcast(F32),
                                        pairs[3][:].bitcast(F32))
                                elif pi == 4:
                                    nc.vector.tensor_add(
                                        accA[:], accA[:].bitcast(F32),
                                        pairs[4][:].bitcast(F32))
                                elif pi == 5:
                                    nc.gpsimd.tensor_add(
                                        accB[:], accB[:],
                                        pairs[5][:].bitcast(F32))
                                elif pi == 6:
                                    nc.vector.tensor_add(
                                        accA[:], accA[:].bitcast(F32),
                                        pairs[6][:].bitcast(F32))
                                    if pend is not None:
                                        norm_colsum(state)
                                        norm_bcast(state)
                                elif pi == 7:
                                    nc.vector.tensor_add(
                                        accA[:], accA[:].bitcast(F32),
                                        pairs[7][:].bitcast(F32))
                                    if pend is not None:
                                        norm_mul(state)
                            state["pend"] = (accA, accB, pso, ctx_t, h)

                    # --- QA schedule --------------------------------------
                    xts = [None] * NSC
                    qTs_l = [None] * NSC
                    ctxs = [None] * NSC
                    state = {"pend": None}
                    nofill = lambda i, st: None

                    xts[0] = load_x(0)
                    xts[1] = load_x(1)
                    # q-projection block for chunk 0 (PE filler: none needed,
                    # the PE is saturated by the block itself)
                    qTs_l[0] = qts.tile([P, HLOC, SCHUNK], BF16, tag="qT", name="qT0")
                    st0 = {}
                    for i in range(MT * DT):
                        qproj_leg(qTs_l[0], xts[0], i, st0)

                    for qc in range(NSC):
                        ctxs[qc] = ctxp.tile([P, HLOC, SCHUNK], BF16, tag="ctx", name="ctx")
                        if qc == 0:
                            # interleave chunk 1's qproj legs mt=0..2
                            xts[2] = load_x(2)
                            qTs_l[1] = qts.tile([P, HLOC, SCHUNK], BF16,
                                                tag="qT", name="qT1")
                            filler = (lambda i, st:
                                      qproj_leg(qTs_l[1], xts[1], i, st)
                                      if i < 48 else None)
                        else:
                            # prologue: finish/run this chunk's qproj, with
                            # the previous chunk's head-3 norm pieces spaced
                            # through it (psden allocated after the second
                            # chain so the op-ring phase stays clean)
                            if qc == 1:
                                xts[3] = load_x(3)
                                for i in range(48, 64):
                                    qproj_leg(qTs_l[1], xts[1], i, state)
                                    if i == 51:
                                        norm_merge(state)
                                    elif i == 54:
                                        norm_colsum(state)
                                    elif i == 58:
                                        norm_bcast(state)
                                    elif i == 61:
                                        norm_mul(state)
                            else:
                                qTs_l[qc] = qts.tile([P, HLOC, SCHUNK], BF16,
                                                     tag="qT", name="qTn")
                                for i in range(MT * DT):
                                    qproj_leg(qTs_l[qc], xts[qc], i, state)
                                    if i == 3:
                                        norm_merge(state)
                                    elif i == DT + 1:
                                        norm_colsum(state)
                                    elif i == DT + 5:
                                        norm_bcast(state)
                                    elif i == DT + 9:
                                        norm_mul(state)
                            pctx = ctxs[qc - 1]
                            pqc = qc - 1
                            filler = (lambda i, st, c=pctx, q=pqc:
                                      oproj_leg(c, q, i, st))
                        h_loop(qc, qTs_l[qc], ctxs[qc], filler, state)

                    # tail: o_proj for the last chunk. Chain 0's jt=0..2
                    # legs need only ctx[0..2], so they cover the last head's
                    # norm pieces; its jt=3 leg follows the multiply.
                    norm_merge(state)
                    st_a = {}
                    for leg in range(MT - 1):
                        oproj_leg(ctxs[NSC - 1], NSC - 1, leg, st_a)
                    norm_colsum(state)
                    norm_bcast(state)
                    norm_mul(state)
                    oproj_leg(ctxs[NSC - 1], NSC - 1, MT - 1, st_a)
                    for i in range(MT, HLOC * KT):
                        oproj_leg(ctxs[NSC - 1], NSC - 1, i, state)

    nc.finalize()
    return nc

_build = build


def _bf16(a):
    return np.ascontiguousarray(a, dtype=np.float32).astype(ml_dtypes.bfloat16)


def kernel(hidden_states, wq, wk, wv, wo):
    global last_run
    if "nc" not in _cache:
        _cache["nc"] = build()
    nc = _cache["nc"]

    hidden_states = np.asarray(hidden_states, dtype=np.float32)
    wq = np.asarray(wq, dtype=np.float32)
    wk = np.asarray(wk, dtype=np.float32)
    wv = np.asarray(wv, dtype=np.float32)
    wo = np.asarray(wo, dtype=np.float32)

    xT = [_bf16(hidden_states[b].T) for b in range(B)]
    in_maps = []
    for c in range(NCORES):
        b, g = divmod(c, G)
        sl = slice(g * DG, (g + 1) * DG)
        in_maps.append({
            "xT": xT[b],
            "wqT": _bf16(wq[sl, :].T),
            "wkT": _bf16(wk[sl, :].T),
            "wvT": _bf16(wv[sl, :].T),
            "woT": _bf16(wo[:, sl].T),
        })

    trace = os.environ.get("BASSKERNEL_TRACE", "0") == "1"
    last_run = run_bass_kernel_spmd(
        nc, in_maps, core_ids=list(range(NCORES)), trace=trace)

    out = np.empty((B, S, D), dtype=np.float32)
    for b in range(B):
        acc = None
        for g in range(G):
            part = last_run.results[b * G + g]["out"]
            acc = part.copy() if acc is None else acc + part
        out[b] = acc
    return out
